# revision 7
# baseline (speedup 1.0000x reference)
"""GatedSSM Trainium2 kernel.

Sharding: TP4 x DP2 over 8 NeuronCores.
  core c: pair p = c//4 owns batches {2p, 2p+1};
          quarter q = c%4 owns state-channels [512*q, 512*(q+1)) of H=2048.
Each core runs the full pipeline for its (batches, channel-quarter):
  RMS-norm (scale folded into weights on host) -> K/u/g_in/g_out projections
  -> sigmoid gating -> first-order linear recurrence (hardware
  tensor_tensor_scan) -> output gate -> out-projection partial.
Host sums the 4 TP partials per batch and adds the residual.

Matmuls run as float32r (full fp32 storage; PE runs it at bf16-rate for
moving dim >= 256) accumulating in fp32 PSUM.
"""
import numpy as np
from contextlib import ExitStack

import concourse.bacc as bacc
import concourse.mybir as mybir
import concourse.tile as tile
from concourse.bass_utils import run_bass_kernel_spmd

B, S, D, H = 4, 2048, 1024, 2048
HQ = H // 4          # channels per core (TP4)
T = 512              # seq chunk
NCHUNK = S // T
F32 = mybir.dt.float32
F32R = mybir.dt.float32r
N_CORES = 8

_CACHED_NC = None


def build_nc():
    """Build + compile the per-core Bass program (same program on all 8 cores)."""
    nc = bacc.Bacc("TRN2", target_bir_lowering=False, debug=False,
                   num_devices=N_CORES)
    xt_h = nc.declare_dram_parameter("xt", [2, D, S], F32R, isOutput=False)
    w_h = nc.declare_dram_parameter("w", [D, 4 * HQ], F32R, isOutput=False)
    wout_h = nc.declare_dram_parameter("wout", [HQ, D], F32R, isOutput=False)
    y_h = nc.declare_dram_parameter("y", [2, S, D], F32, isOutput=True)

    xt = xt_h.ap()
    w = w_h.ap()
    wout = wout_h.ap()
    y = y_h.ap()

    with tile.TileContext(nc) as tc, ExitStack() as ctx, \
            nc.allow_low_precision(reason="fp32r matmul operand tiles"):
        singles = ctx.enter_context(tc.tile_pool(name="singles", bufs=1))
        xp = ctx.enter_context(tc.tile_pool(name="xp", bufs=2))
        sqp = ctx.enter_context(tc.tile_pool(name="sqp", bufs=3))
        normp = ctx.enter_context(tc.tile_pool(name="normp", bufs=2))
        gatep = ctx.enter_context(tc.tile_pool(name="gatep", bufs=2))
        gshared = ctx.enter_context(tc.tile_pool(name="gshared", bufs=3))
        sop = ctx.enter_context(tc.tile_pool(name="sop", bufs=2))
        carryp = ctx.enter_context(tc.tile_pool(name="carryp", bufs=2))
        ybp = ctx.enter_context(tc.tile_pool(name="ybp", bufs=3))
        ps_n = ctx.enter_context(tc.tile_pool(name="ps_n", bufs=2, space="PSUM"))
        ps_b = ctx.enter_context(tc.tile_pool(name="ps_b", bufs=2, space="PSUM"))
        ps_p = ctx.enter_context(tc.tile_pool(name="ps_p", bufs=2, space="PSUM"))
        ps_y = ctx.enter_context(tc.tile_pool(name="ps_y", bufs=2, space="PSUM"))

        # resident weights
        wsb = singles.tile([128, 8, 4 * HQ], F32R)          # [d_lo, d_hi, hcol]
        nc.sync.dma_start(out=wsb[:], in_=w.rearrange("(dh dl) h -> dl dh h", dl=128))
        wosb = singles.tile([128, HQ // 128, D], F32R)      # [h_lo, h_hi, d]
        nc.sync.dma_start(out=wosb[:], in_=wout.rearrange("(hh hl) d -> hl hh d", hl=128))
        ones_col_f = singles.tile([128, 1], F32)
        nc.vector.memset(ones_col_f[:], 1.0)
        ones_col = singles.tile([128, 1], F32R)
        nc.vector.tensor_copy(ones_col[:], ones_col_f[:])
        ones_row_f = singles.tile([1, 128], F32)
        nc.vector.memset(ones_row_f[:], 1.0)
        ones_row = singles.tile([1, 128], F32R)
        nc.vector.tensor_copy(ones_row[:], ones_row_f[:])

        NS = HQ // 128    # 4 channel slices per group
        # group column offsets inside w: [K | u | g_in | g_out]
        OFF_K, OFF_U, OFF_GI, OFF_GO = 0, HQ, 2 * HQ, 3 * HQ

        for bl in range(2):
            prev_kbuf = [None] * NS
            prev_carry = [None] * NS
            for c in range(NCHUNK):
                t0 = c * T
                # ---- load x^T chunk [128, 8, T] ----
                xsb = xp.tile([128, 8, T], F32R, tag="xsb")
                nc.sync.dma_start(
                    out=xsb[:],
                    in_=xt[bl].rearrange("(dh dl) t -> dl dh t", dl=128)[:, :, t0:t0 + T],
                )
                # ---- rms norm: sumsq over D via ones-matmul ----
                nsum = ps_n.tile([1, T], F32, tag="nsum")
                for dh in range(8):
                    sq = sqp.tile([128, T], F32R, tag="sq")
                    nc.vector.tensor_mul(sq[:], xsb[:, dh, :], xsb[:, dh, :])
                    nc.tensor.matmul(nsum[:], ones_col[:], sq[:],
                                     start=(dh == 0), stop=(dh == 7))
                norm = normp.tile([1, T], F32, tag="norm")
                nc.scalar.activation(out=norm[:], in_=nsum[:],
                                     func=mybir.ActivationFunctionType.Sqrt,
                                     scale=1.0 / D)
                nc.vector.tensor_scalar_add(norm[:], norm[:], 1e-8)
                inv = normp.tile([1, T], F32R, tag="inv")
                nc.vector.reciprocal(out=inv[:], in_=norm[:])
                # broadcast inv over 128 partitions via rank-1 matmul
                bc = ps_b.tile([128, T], F32, tag="bc")
                nc.tensor.matmul(bc[:], ones_row[:], inv[:],
                                 start=True, stop=True)
                # xn = x * inv (in place)
                for dh in range(8):
                    nc.vector.tensor_mul(xsb[:, dh, :], xsb[:, dh, :], bc[:])

                # ---- projections + gating + scan, per channel slice ----
                so_tiles = []
                for s in range(NS):
                    # K slice
                    psK = ps_p.tile([128, T], F32, tag="pp")
                    for dh in range(8):
                        nc.tensor.matmul(psK[:], wsb[:, dh, OFF_K + 128 * s:OFF_K + 128 * (s + 1)],
                                         xsb[:, dh, :], start=(dh == 0), stop=(dh == 7))
                    kbuf = gatep.tile([128, T + 1], F32, tag=f"kb{s}")
                    nc.scalar.activation(out=kbuf[:, 1:T + 1], in_=psK[:],
                                         func=mybir.ActivationFunctionType.Sigmoid)
                    km1 = gshared.tile([128, T], F32, tag="km1")
                    nc.scalar.activation(out=km1[:], in_=psK[:],
                                         func=mybir.ActivationFunctionType.Sigmoid,
                                         scale=-1.0)
                    if c == 0:
                        nc.vector.memset(kbuf[:, 0:1], 0.0)
                    else:
                        nc.vector.tensor_copy(kbuf[:, 0:1], prev_kbuf[s][:, T:T + 1])
                    # g_in slice
                    psGi = ps_p.tile([128, T], F32, tag="pp")
                    for dh in range(8):
                        nc.tensor.matmul(psGi[:], wsb[:, dh, OFF_GI + 128 * s:OFF_GI + 128 * (s + 1)],
                                         xsb[:, dh, :], start=(dh == 0), stop=(dh == 7))
                    gi = gshared.tile([128, T], F32, tag="gi")
                    nc.scalar.activation(out=gi[:], in_=psGi[:],
                                         func=mybir.ActivationFunctionType.Sigmoid)
                    # u slice
                    psU = ps_p.tile([128, T], F32, tag="pp")
                    for dh in range(8):
                        nc.tensor.matmul(psU[:], wsb[:, dh, OFF_U + 128 * s:OFF_U + 128 * (s + 1)],
                                         xsb[:, dh, :], start=(dh == 0), stop=(dh == 7))
                    ueff = gshared.tile([128, T], F32, tag="ue")
                    nc.vector.tensor_mul(ueff[:], psU[:], gi[:])
                    nc.vector.tensor_mul(ueff[:], ueff[:], km1[:])
                    # g_out slice
                    psGo = ps_p.tile([128, T], F32, tag="pp")
                    for dh in range(8):
                        nc.tensor.matmul(psGo[:], wsb[:, dh, OFF_GO + 128 * s:OFF_GO + 128 * (s + 1)],
                                         xsb[:, dh, :], start=(dh == 0), stop=(dh == 7))
                    go = gshared.tile([128, T], F32, tag="go")
                    nc.scalar.activation(out=go[:], in_=psGo[:],
                                         func=mybir.ActivationFunctionType.Sigmoid)
                    # scan: state = kshift[t]*state + u[t]
                    so = sop.tile([128, T], F32R, tag=f"so{s}")
                    init = 0.0 if c == 0 else prev_carry[s][:, 0:1]
                    nc.vector.tensor_tensor_scan(
                        out=so[:], data0=kbuf[:, 0:T], data1=ueff[:],
                        initial=init,
                        op0=mybir.AluOpType.mult, op1=mybir.AluOpType.add)
                    carry = carryp.tile([128, 1], F32R, tag=f"ca{s}")
                    nc.vector.tensor_copy(carry[:], so[:, T - 1:T])
                    # v = scan_out * sigmoid(g_out)  (in place over so)
                    nc.vector.tensor_mul(so[:], so[:], go[:])
                    prev_kbuf[s] = kbuf
                    prev_carry[s] = carry
                    so_tiles.append(so)

                # ---- out-projection: y[t0+tt*128 : .., :] partials ----
                for tt in range(T // 128):
                    yb = ybp.tile([128, D], F32, tag="yb")
                    for dcol in range(2):
                        psY = ps_y.tile([128, 512], F32, tag="py")
                        for s in range(NS):
                            nc.tensor.matmul(
                                psY[:],
                                so_tiles[s][:, tt * 128:(tt + 1) * 128],
                                wosb[:, s, 512 * dcol:512 * (dcol + 1)],
                                start=(s == 0), stop=(s == NS - 1))
                        nc.vector.tensor_copy(yb[:, 512 * dcol:512 * (dcol + 1)], psY[:])
                    nc.sync.dma_start(
                        out=y[bl, t0 + tt * 128:t0 + (tt + 1) * 128, :],
                        in_=yb[:])

    nc.compile()
    return nc


def _get_nc():
    global _CACHED_NC
    if _CACHED_NC is None:
        _CACHED_NC = build_nc()
    return _CACHED_NC


def prep_in_maps(x, rms_scale, split_scale, W_K, W_ugg, W_out):
    s = (rms_scale.astype(np.float32) * split_scale.astype(np.float32))
    xt = np.ascontiguousarray(x.transpose(0, 2, 1), dtype=np.float32)  # [B, D, S]
    in_maps = []
    for c in range(N_CORES):
        pair, q = c // 4, c % 4
        cols = [W_K[:, q * HQ:(q + 1) * HQ],
                W_ugg[:, q * HQ:(q + 1) * HQ],
                W_ugg[:, H + q * HQ:H + (q + 1) * HQ],
                W_ugg[:, 2 * H + q * HQ:2 * H + (q + 1) * HQ]]
        Wq = np.ascontiguousarray(
            np.concatenate(cols, axis=1) * s[:, None], dtype=np.float32)
        Wo = np.ascontiguousarray(W_out[q * HQ:(q + 1) * HQ, :], dtype=np.float32)
        in_maps.append({
            "xt": np.ascontiguousarray(xt[2 * pair:2 * pair + 2]),
            "w": Wq,
            "wout": Wo,
        })
    return in_maps


def gather_out(x, results):
    y = np.zeros_like(x, dtype=np.float32)
    for c in range(N_CORES):
        pair = c // 4
        y[2 * pair:2 * pair + 2] += results[c]["y"]
    return y + x


def kernel(x, rms_scale, split_scale, W_K, W_ugg, W_out):
    nc = _get_nc()
    in_maps = prep_in_maps(x, rms_scale, split_scale, W_K, W_ugg, W_out)
    res = run_bass_kernel_spmd(nc, in_maps, list(range(N_CORES)))
    return gather_out(x, res.results)


# revision 9
# speedup vs baseline: 52.9401x; 52.9401x over previous
"""GatedSSM Trainium2 kernel.

Sharding: TP4 x DP2 over 8 NeuronCores.
  core c: pair p = c//4 owns batches {2p, 2p+1};
          quarter q = c%4 owns state-channels [512*q, 512*(q+1)) of H=2048.
Each core runs the full pipeline for its (batches, channel-quarter):
  RMS-norm (scale folded into weights on host) -> K/u/g_in/g_out projections
  -> sigmoid gating -> first-order linear recurrence (hardware
  tensor_tensor_scan) -> output gate -> out-projection partial.
Host sums the 4 TP partials per batch and adds the residual.

Matmuls run as float32r (full fp32 storage; PE runs it at bf16-rate for
moving dim >= 256) accumulating in fp32 PSUM.
"""
import numpy as np
from contextlib import ExitStack

import concourse.bacc as bacc
import concourse.mybir as mybir
import concourse.tile as tile
from concourse.bass_utils import run_bass_kernel_spmd

B, S, D, H = 4, 2048, 1024, 2048
HQ = H // 4          # channels per core (TP4)
T = 512              # seq chunk
NCHUNK = S // T
F32 = mybir.dt.float32
F32R = mybir.dt.float32r
N_CORES = 8

_CACHED_NC = None


def build_nc(repeat: int = 1):
    """Build + compile the per-core Bass program (same program on all 8 cores).

    repeat > 1 wraps the whole body in a hardware loop that recomputes the
    identical result `repeat` times — used only for device-time measurement
    (amortizes host/RPC dispatch overhead out of the timing).
    """
    nc = bacc.Bacc("TRN2", target_bir_lowering=False, debug=False,
                   num_devices=N_CORES)
    xt_h = nc.declare_dram_parameter("xt", [2, D, S], F32R, isOutput=False)
    w_h = nc.declare_dram_parameter("w", [D, 4 * HQ], F32R, isOutput=False)
    wout_h = nc.declare_dram_parameter("wout", [HQ, D], F32R, isOutput=False)
    y_h = nc.declare_dram_parameter("y", [2, S, D], F32, isOutput=True)

    xt = xt_h.ap()
    w = w_h.ap()
    wout = wout_h.ap()
    y = y_h.ap()

    with tile.TileContext(nc) as tc, ExitStack() as ctx, \
            nc.allow_low_precision(reason="fp32r matmul operand tiles"):
        singles = ctx.enter_context(tc.tile_pool(name="singles", bufs=1))
        xp = ctx.enter_context(tc.tile_pool(name="xp", bufs=2))
        sqp = ctx.enter_context(tc.tile_pool(name="sqp", bufs=3))
        normp = ctx.enter_context(tc.tile_pool(name="normp", bufs=2))
        gatep = ctx.enter_context(tc.tile_pool(name="gatep", bufs=2))
        gshared = ctx.enter_context(tc.tile_pool(name="gshared", bufs=3))
        sop = ctx.enter_context(tc.tile_pool(name="sop", bufs=2))
        carryp = ctx.enter_context(tc.tile_pool(name="carryp", bufs=2))
        ybp = ctx.enter_context(tc.tile_pool(name="ybp", bufs=3))
        ps_n = ctx.enter_context(tc.tile_pool(name="ps_n", bufs=2, space="PSUM"))
        ps_p = ctx.enter_context(tc.tile_pool(name="ps_p", bufs=4, space="PSUM"))
        ps_y = ctx.enter_context(tc.tile_pool(name="ps_y", bufs=2, space="PSUM"))

        # resident weights — on the scalar-engine HWDGE ring, split into
        # per-subtile pieces so the first projection matmuls can start as
        # soon as their slice lands (x loads use the sync ring in parallel)
        wsb = singles.tile([128, 8, 4 * HQ], F32R)          # [d_lo, d_hi, hcol]
        w_r = w.rearrange("(dh dl) h -> dl dh h", dl=128)
        for dh in range(8):
            nc.scalar.dma_start(out=wsb[:, dh, :], in_=w_r[:, dh, :])
        wosb = singles.tile([128, HQ // 128, D], F32R)      # [h_lo, h_hi, d]
        wo_r = wout.rearrange("(hh hl) d -> hl hh d", hl=128)
        for hh in range(HQ // 128):
            nc.scalar.dma_start(out=wosb[:, hh, :], in_=wo_r[:, hh, :])
        ones_col_f = singles.tile([128, 1], F32)
        nc.vector.memset(ones_col_f[:], 1.0)
        ones_col = singles.tile([128, 1], F32R)
        nc.vector.tensor_copy(ones_col[:], ones_col_f[:])
        ones_row_f = singles.tile([1, 128], F32)
        nc.vector.memset(ones_row_f[:], 1.0)
        ones_row = singles.tile([1, 128], F32R)
        nc.vector.tensor_copy(ones_row[:], ones_row_f[:])

        NS = HQ // 128    # 4 channel slices per group
        # group column offsets inside w: [K | u | g_in | g_out]
        OFF_K, OFF_U, OFF_GI, OFF_GO = 0, HQ, 2 * HQ, 3 * HQ

        loop_cm = tc.For_i(0, repeat, 1) if repeat > 1 else ExitStack()
        ctx.enter_context(loop_cm)
        for bl in range(2):
            prev_kbuf = [None] * NS
            prev_carry = [None] * NS
            for c in range(NCHUNK):
                t0 = c * T
                # ---- load x^T chunk [128, 8, T] ----
                xsb = xp.tile([128, 8, T], F32R, tag="xsb")
                nc.sync.dma_start(
                    out=xsb[:],
                    in_=xt[bl].rearrange("(dh dl) t -> dl dh t", dl=128)[:, :, t0:t0 + T],
                )
                # ---- rms norm: sumsq over D via ones-matmul ----
                nsum = ps_n.tile([1, T], F32, tag="nsum")
                for dh in range(8):
                    sq = sqp.tile([128, T], F32R, tag="sq")
                    nc.scalar.square(sq[:], xsb[:, dh, :])
                    nc.tensor.matmul(nsum[:], ones_col[:], sq[:],
                                     start=(dh == 0), stop=(dh == 7))
                norm = normp.tile([1, T], F32, tag="norm")
                nc.scalar.activation(out=norm[:], in_=nsum[:],
                                     func=mybir.ActivationFunctionType.Sqrt,
                                     scale=1.0 / D)
                nc.vector.tensor_scalar_add(norm[:], norm[:], 1e-8)
                inv = normp.tile([1, T], F32R, tag="inv")
                nc.vector.reciprocal(out=inv[:], in_=norm[:])
                # broadcast inv over 128 partitions via rank-1 matmul
                bc = ps_n.tile([128, T], F32, tag="nsum")
                nc.tensor.matmul(bc[:], ones_row[:], inv[:],
                                 start=True, stop=True)
                # xn = x * inv (in place)
                for dh in range(8):
                    nc.vector.tensor_mul(xsb[:, dh, :], xsb[:, dh, :], bc[:])

                # ---- projections + gating + scan, per channel slice ----
                so_tiles = []
                for s in range(NS):
                    # K slice
                    psK = ps_p.tile([128, T], F32, tag="pp")
                    for dh in range(8):
                        nc.tensor.matmul(psK[:], wsb[:, dh, OFF_K + 128 * s:OFF_K + 128 * (s + 1)],
                                         xsb[:, dh, :], start=(dh == 0), stop=(dh == 7))
                    kbuf = gatep.tile([128, T + 1], F32, tag=f"kb{s}")
                    nc.scalar.activation(out=kbuf[:, 1:T + 1], in_=psK[:],
                                         func=mybir.ActivationFunctionType.Sigmoid)
                    km1 = gshared.tile([128, T], F32, tag="km1")
                    nc.scalar.activation(out=km1[:], in_=psK[:],
                                         func=mybir.ActivationFunctionType.Sigmoid,
                                         scale=-1.0)
                    if c == 0:
                        nc.vector.memset(kbuf[:, 0:1], 0.0)
                    else:
                        nc.vector.tensor_copy(kbuf[:, 0:1], prev_kbuf[s][:, T:T + 1])
                    # g_in slice
                    psGi = ps_p.tile([128, T], F32, tag="pp")
                    for dh in range(8):
                        nc.tensor.matmul(psGi[:], wsb[:, dh, OFF_GI + 128 * s:OFF_GI + 128 * (s + 1)],
                                         xsb[:, dh, :], start=(dh == 0), stop=(dh == 7))
                    gi = gshared.tile([128, T], F32, tag="gi")
                    nc.scalar.activation(out=gi[:], in_=psGi[:],
                                         func=mybir.ActivationFunctionType.Sigmoid)
                    # u slice
                    psU = ps_p.tile([128, T], F32, tag="pp")
                    for dh in range(8):
                        nc.tensor.matmul(psU[:], wsb[:, dh, OFF_U + 128 * s:OFF_U + 128 * (s + 1)],
                                         xsb[:, dh, :], start=(dh == 0), stop=(dh == 7))
                    ueff = gshared.tile([128, T], F32, tag="ue")
                    nc.vector.tensor_mul(ueff[:], psU[:], gi[:])
                    nc.gpsimd.tensor_mul(ueff[:], ueff[:], km1[:])
                    # g_out slice
                    psGo = ps_p.tile([128, T], F32, tag="pp")
                    for dh in range(8):
                        nc.tensor.matmul(psGo[:], wsb[:, dh, OFF_GO + 128 * s:OFF_GO + 128 * (s + 1)],
                                         xsb[:, dh, :], start=(dh == 0), stop=(dh == 7))
                    go = gshared.tile([128, T], F32, tag="go")
                    nc.scalar.activation(out=go[:], in_=psGo[:],
                                         func=mybir.ActivationFunctionType.Sigmoid)
                    # scan: state = kshift[t]*state + u[t]
                    so = sop.tile([128, T], F32R, tag=f"so{s}")
                    init = 0.0 if c == 0 else prev_carry[s][:, 0:1]
                    nc.vector.tensor_tensor_scan(
                        out=so[:], data0=kbuf[:, 0:T], data1=ueff[:],
                        initial=init,
                        op0=mybir.AluOpType.mult, op1=mybir.AluOpType.add)
                    carry = carryp.tile([128, 1], F32R, tag=f"ca{s}")
                    nc.vector.tensor_copy(carry[:], so[:, T - 1:T])
                    # v = scan_out * sigmoid(g_out)  (in place over so)
                    nc.vector.tensor_mul(so[:], so[:], go[:])
                    prev_kbuf[s] = kbuf
                    prev_carry[s] = carry
                    so_tiles.append(so)

                # ---- out-projection: y[t0+tt*128 : .., :] partials ----
                for tt in range(T // 128):
                    yb = ybp.tile([128, D], F32, tag="yb")
                    for dcol in range(2):
                        psY = ps_y.tile([128, 512], F32, tag="py")
                        for s in range(NS):
                            nc.tensor.matmul(
                                psY[:],
                                so_tiles[s][:, tt * 128:(tt + 1) * 128],
                                wosb[:, s, 512 * dcol:512 * (dcol + 1)],
                                start=(s == 0), stop=(s == NS - 1))
                        nc.vector.tensor_copy(yb[:, 512 * dcol:512 * (dcol + 1)], psY[:])
                    nc.sync.dma_start(
                        out=y[bl, t0 + tt * 128:t0 + (tt + 1) * 128, :],
                        in_=yb[:])

    nc.compile()
    return nc


def _get_nc():
    global _CACHED_NC
    if _CACHED_NC is None:
        _CACHED_NC = build_nc()
    return _CACHED_NC


def prep_in_maps(x, rms_scale, split_scale, W_K, W_ugg, W_out):
    s = (rms_scale.astype(np.float32) * split_scale.astype(np.float32))
    xt = np.ascontiguousarray(x.transpose(0, 2, 1), dtype=np.float32)  # [B, D, S]
    in_maps = []
    for c in range(N_CORES):
        pair, q = c // 4, c % 4
        cols = [W_K[:, q * HQ:(q + 1) * HQ],
                W_ugg[:, q * HQ:(q + 1) * HQ],
                W_ugg[:, H + q * HQ:H + (q + 1) * HQ],
                W_ugg[:, 2 * H + q * HQ:2 * H + (q + 1) * HQ]]
        Wq = np.ascontiguousarray(
            np.concatenate(cols, axis=1) * s[:, None], dtype=np.float32)
        Wo = np.ascontiguousarray(W_out[q * HQ:(q + 1) * HQ, :], dtype=np.float32)
        in_maps.append({
            "xt": np.ascontiguousarray(xt[2 * pair:2 * pair + 2]),
            "w": Wq,
            "wout": Wo,
        })
    return in_maps


def gather_out(x, results):
    y = np.zeros_like(x, dtype=np.float32)
    for c in range(N_CORES):
        pair = c // 4
        y[2 * pair:2 * pair + 2] += results[c]["y"]
    return y + x


def kernel(x, rms_scale, split_scale, W_K, W_ugg, W_out):
    nc = _get_nc()
    in_maps = prep_in_maps(x, rms_scale, split_scale, W_K, W_ugg, W_out)
    res = run_bass_kernel_spmd(nc, in_maps, list(range(N_CORES)))
    return gather_out(x, res.results)
